# revision 3
# baseline (speedup 1.0000x reference)
"""Bi-tempered weighted logistic loss on 8 Trainium2 NeuronCores.

Strategy (data-parallel over the batch, per the sharding hint):
  - Each of the 8 cores gets a [4096, 1000] shard of the logits.
  - Per 128-row tile the device solves for the tempered-softmax normalizer
    lambda (the fixed point of the reference's compute_normalization) by
    root-finding on F(lam) = sum_j x_j^-5 - 1 with x = 1 - 0.2*(logit-lam):
        eval0 at lam=mu (row max)  -> lp0 = sum x^-5, m6 = sum x^-6
        "spike jump":  model lp(d) = A*(B+0.2d)^-5 fitted to (lp0, m6)
        eval1 at jumped lam        -> lp1
        secant step in g = lp^-0.2 space (g is nearly linear in lam)
    All heavy elementwise work is ScalarE Ln/Exp passes (one table set);
    reductions ride the activation accumulator.  The final pass emits the
    two weighted moments the loss needs:
        A = sum_j pw_j * x_j^-1      B = sum_j pw_j * x_j^-6
  - Host (numpy, float64) assembles the closed-form loss from lambda, A, B,
    plus the one-hot terms via cheap gathers, and averages over the batch.

Numerics: the reference's 5-iteration fixed point is converged to ~5e-3 in
lambda but the loss is insensitive (dLoss/dlam ~ 0.06); this scheme lands
within ~3e-6 relative of the reference loss (validated in fp32 simulation).
"""

import numpy as np

import concourse.bass as bass
import concourse.mybir as mybir
import concourse.tile as tile
from concourse import bacc
from concourse.bass_utils import run_bass_kernel_spmd

# Problem constants (hardcoded: kernel.py must be self-contained).
B_FULL, C = 32768, 1000
N_CORES = 8
B_SHARD = B_FULL // N_CORES  # 4096
P = 128
NT = B_SHARD // P  # 32 tiles per core
T1, T2, SMOOTHING = 0.8, 1.2, 0.05

F32 = mybir.dt.float32
AX = mybir.AxisListType
OP = mybir.AluOpType
AF = mybir.ActivationFunctionType


def _build_program():
    nc = bacc.Bacc("TRN2", debug=False, target_bir_lowering=False,
                   enable_asserts=False)
    logit = nc.dram_tensor("logit", [B_SHARD, C], F32, kind="ExternalInput").ap()
    lnpw = nc.dram_tensor("lnpw", [P, C], F32, kind="ExternalInput").ap()
    stats = nc.dram_tensor("stats", [P, 4 * NT], F32, kind="ExternalOutput").ap()

    with tile.TileContext(nc) as tc:
        with (
            tc.tile_pool(name="const", bufs=1) as const,
            tc.tile_pool(name="lg", bufs=4) as lg,
            tc.tile_pool(name="tln", bufs=3) as tln,
            tc.tile_pool(name="ej", bufs=2) as ej,
            tc.tile_pool(name="eap", bufs=2) as eap,
            tc.tile_pool(name="e5fp", bufs=2) as e5fp,
            tc.tile_pool(name="v1p", bufs=2) as v1p,
            tc.tile_pool(name="tjp", bufs=2) as tjp,
            tc.tile_pool(name="sm", bufs=6) as sm,
        ):
            lnpw_t = const.tile([P, C], F32, tag="lnpw")
            nc.sync.dma_start(lnpw_t[:], lnpw[:, :])
            stage = const.tile([P, 4 * NT], F32, tag="stage")

            def small(tag):
                return sm.tile([P, 1], F32, tag=tag, name=tag)

            for i in range(NT):
                T = lg.tile([P, C], F32, tag="T")
                nc.sync.dma_start(T[:], logit[i * P:(i + 1) * P, :])

                mu = small("mu")
                nc.vector.tensor_reduce(mu[:], T[:], axis=AX.X, op=OP.max)
                bias0 = small("bias0")
                nc.vector.tensor_scalar(bias0[:], mu[:], 0.2, 1.0, OP.mult, OP.add)

                # ---- eval 0 at lam = mu ----
                t0 = tln.tile([P, C], F32, tag="t")
                nc.scalar.activation(t0[:], T[:], AF.Ln, bias=bias0[:], scale=-0.2)
                lp0 = small("lp0")
                e5 = ej.tile([P, C], F32, tag="ej")
                nc.scalar.activation(e5[:], t0[:], AF.Exp, scale=-5.0,
                                     accum_out=lp0[:])
                m6 = small("m6")
                e6 = ej.tile([P, C], F32, tag="ej")
                nc.scalar.activation(e6[:], t0[:], AF.Exp, scale=-6.0,
                                     accum_out=m6[:])

                # ---- spike jump: bias1 = bias0 + lp0^0.2*Bc - Bc, Bc = lp0/m6
                rm6 = small("rm6")
                nc.vector.reciprocal(rm6[:], m6[:])
                Bc = small("Bc")
                nc.vector.tensor_mul(Bc[:], lp0[:], rm6[:])
                lnlp0 = small("lnlp0")
                nc.scalar.activation(lnlp0[:], lp0[:], AF.Ln)
                g0 = small("g0")
                nc.scalar.activation(g0[:], lnlp0[:], AF.Exp, scale=-0.2)
                rg0 = small("rg0")
                nc.vector.reciprocal(rg0[:], g0[:])
                A15 = small("A15")  # lp0^0.2 * Bc  (== Bc / g0)
                nc.vector.tensor_mul(A15[:], Bc[:], rg0[:])
                d0 = small("d0")
                nc.vector.tensor_sub(d0[:], A15[:], Bc[:])
                bias1 = small("bias1")
                nc.vector.tensor_add(bias1[:], bias0[:], d0[:])

                # ---- eval 1 at jumped lambda ----
                t1 = tln.tile([P, C], F32, tag="t")
                nc.scalar.activation(t1[:], T[:], AF.Ln, bias=bias1[:], scale=-0.2)
                lp1 = small("lp1")
                e5b = ej.tile([P, C], F32, tag="ej")
                nc.scalar.activation(e5b[:], t1[:], AF.Exp, scale=-5.0,
                                     accum_out=lp1[:])
                lnlp1 = small("lnlp1")
                nc.scalar.activation(lnlp1[:], lp1[:], AF.Ln)
                g1 = small("g1")
                nc.scalar.activation(g1[:], lnlp1[:], AF.Exp, scale=-0.2)

                # ---- secant in g-space: bias2 = bias1 + (1-g1)*(bias1-bias0)/(g1-g0)
                den = small("den")
                nc.vector.tensor_sub(den[:], g1[:], g0[:])
                denc = small("denc")
                nc.vector.tensor_scalar_max(denc[:], den[:], 1e-12)
                rden = small("rden")
                nc.vector.reciprocal(rden[:], denc[:])
                num = small("num")
                nc.vector.tensor_sub(num[:], bias1[:], bias0[:])
                w1 = small("w1")
                nc.vector.tensor_scalar(w1[:], g1[:], -1.0, 1.0, OP.mult, OP.add)
                p1 = small("p1")
                nc.vector.tensor_mul(p1[:], w1[:], num[:])
                d1 = small("d1")
                nc.vector.tensor_mul(d1[:], p1[:], rden[:])
                bias2 = small("bias2")
                nc.vector.tensor_add(bias2[:], bias1[:], d1[:])

                # ---- final pass at bias2 ----
                t2 = tln.tile([P, C], F32, tag="t")
                nc.scalar.activation(t2[:], T[:], AF.Ln, bias=bias2[:], scale=-0.2)
                v1 = v1p.tile([P, C], F32, tag="v1")  # lnpw - t2
                nc.vector.scalar_tensor_tensor(v1[:], t2[:], -1.0, lnpw_t[:],
                                               OP.mult, OP.add)
                Asum = small("Asum")
                eA = eap.tile([P, C], F32, tag="eA")
                nc.scalar.activation(eA[:], v1[:], AF.Exp, accum_out=Asum[:])
                lpf = small("lpf")
                e5f = e5fp.tile([P, C], F32, tag="e5f")
                nc.scalar.activation(e5f[:], t2[:], AF.Exp, scale=-5.0,
                                     accum_out=lpf[:])
                Bsum = small("Bsum")
                tjk = tjp.tile([P, C], F32, tag="tjk")
                nc.vector.scalar_tensor_tensor(tjk[:], eA[:], 1.0, e5f[:],
                                               OP.mult, OP.mult,
                                               accum_out=Bsum[:])

                nc.vector.tensor_copy(stage[:, i:i + 1], bias2[:])
                nc.vector.tensor_copy(stage[:, NT + i:NT + i + 1], Asum[:])
                nc.vector.tensor_copy(stage[:, 2 * NT + i:2 * NT + i + 1], Bsum[:])
                nc.vector.tensor_copy(stage[:, 3 * NT + i:3 * NT + i + 1], lpf[:])

            nc.sync.dma_start(stats[:, :], stage[:])

    nc.compile()
    return nc


_PROGRAM = None


def _get_program():
    global _PROGRAM
    if _PROGRAM is None:
        _PROGRAM = _build_program()
    return _PROGRAM


def _run_device(logit_f32, lnpw_rep, trace=False):
    nc = _get_program()
    shards = logit_f32.reshape(N_CORES, B_SHARD, C)
    in_maps = [
        {"logit": np.ascontiguousarray(shards[c]), "lnpw": lnpw_rep}
        for c in range(N_CORES)
    ]
    return run_bass_kernel_spmd(nc, in_maps, list(range(N_CORES)), trace=trace)


def _assemble(results, logit_f32, truth, pw):
    """Host-side finish in float64 from per-row (lambda, A, B)."""
    bias_f = np.empty((N_CORES, P, NT), np.float64)
    A = np.empty((N_CORES, P, NT), np.float64)
    Bm = np.empty((N_CORES, P, NT), np.float64)
    for c in range(N_CORES):
        st = results[c]["stats"].astype(np.float64)  # [P, 4*NT]
        bias_f[c] = st[:, 0:NT]
        A[c] = st[:, NT:2 * NT]
        Bm[c] = st[:, 2 * NT:3 * NT]
    # row r of shard c = tile i, partition p  ->  index [c, p, i]
    perm = (0, 2, 1)  # -> [c, i, p]
    bias_f = bias_f.transpose(perm).reshape(B_FULL)
    A = A.transpose(perm).reshape(B_FULL)
    Bm = Bm.transpose(perm).reshape(B_FULL)
    lam = (bias_f - 1.0) * 5.0

    c_off = SMOOTHING / (C - 1)
    c_on = (1.0 - SMOOTHING * C / (C - 1)) + c_off

    def log_t1(u):
        return (u ** (1.0 - T1) - 1.0) / (1.0 - T1)

    def f_y(y):
        return y * log_t1(y + 1e-10) - y ** (2.0 - T1) / (2.0 - T1)

    f_off, f_on = f_y(c_off), f_y(c_on)
    pwk = pw[truth]
    glk = logit_f32.astype(np.float64)[np.arange(B_FULL), truth]
    x_k = 1.0 - 0.2 * (glk - lam)
    loss_rows = (
        C * f_off + (f_on - f_off) * pwk
        + 5.0 * (c_off * C + (c_on - c_off) * pwk)
        - 5.0 * (c_off * A + (c_on - c_off) * pwk / x_k)
        + Bm / 1.2
    )
    return np.float32(loss_rows.mean())


def kernel(logit_label, truth_label, weight):
    logit_f32 = np.ascontiguousarray(np.asarray(logit_label, dtype=np.float32))
    truth = np.asarray(truth_label).astype(np.int64)
    w = np.asarray(weight, dtype=np.float64)
    pw = w / w.sum() * C
    lnpw_rep = np.ascontiguousarray(
        np.broadcast_to(np.log(pw).astype(np.float32), (P, C))
    )
    res = _run_device(logit_f32, lnpw_rep, trace=False)
    return _assemble(res.results, logit_f32, truth, pw)


# revision 5
# speedup vs baseline: 1.6543x; 1.6543x over previous
"""Bi-tempered weighted logistic loss on 8 Trainium2 NeuronCores.

Strategy (data-parallel over the batch, per the sharding hint):
  - Each of the 8 cores gets a [4096, 1000] shard of the logits.
  - Per 128-row tile the device solves for the tempered-softmax normalizer
    lambda (the fixed point of the reference's compute_normalization) by
    root-finding on F(lam) = sum_j x_j^-5 - 1 with x = 1 - 0.2*(logit-lam):
        eval0 at lam=mu (row max)  -> lp0 = sum x^-5, m6 = sum x^-6
        "spike jump":  model lp(d) = A*(B+0.2d)^-5 fitted to (lp0, m6)
        eval1 at jumped lam        -> lp1
        secant step in g = lp^-0.2 space (g is nearly linear in lam)
    All heavy elementwise work is ScalarE Ln/Exp passes (one table set);
    reductions ride the activation accumulator.  The final pass emits the
    two weighted moments the loss needs:
        A = sum_j pw_j * x_j^-1      B = sum_j pw_j * x_j^-6
  - Host (numpy, float64) assembles the closed-form loss from lambda, A, B,
    plus the one-hot terms via cheap gathers, and averages over the batch.

Numerics: the reference's 5-iteration fixed point is converged to ~5e-3 in
lambda but the loss is insensitive (dLoss/dlam ~ 0.06); this scheme lands
within ~3e-6 relative of the reference loss (validated in fp32 simulation).
"""

import numpy as np

import concourse.bass as bass
import concourse.mybir as mybir
import concourse.tile as tile
from concourse import bacc
from concourse.bass_utils import run_bass_kernel_spmd

# Problem constants (hardcoded: kernel.py must be self-contained).
B_FULL, C = 32768, 1000
N_CORES = 8
B_SHARD = B_FULL // N_CORES  # 4096
P = 128
NT = B_SHARD // P  # 32 tiles per core
T1, T2, SMOOTHING = 0.8, 1.2, 0.05

F32 = mybir.dt.float32
AX = mybir.AxisListType
OP = mybir.AluOpType
AF = mybir.ActivationFunctionType

_COMBINED_SET = "natural_log_exp_and_others"
_TABLES_PATCHED = False


def _patch_act_tables():
    """Make Ln/Exp resolvable only via the combined ln+exp table set.

    The act-table-load insertion pass picks the first set containing each
    activation's function; with Ln and Exp interleaved it flip-flops between
    the exp-only and ln-only sets, inserting a ~1.3us ACT_TABLE_LOAD before
    almost every ACTIVATE (measured 258 loads = 331us, half the kernel).
    Removing Ln/Exp from every other set (indices preserved) pins both
    functions to one set, so the fixpoint inserts a single load.
    """
    global _TABLES_PATCHED
    if _TABLES_PATCHED:
        return
    import concourse.hw_specs as hw_specs
    orig = hw_specs.get_activation_tables

    def patched(module_arch):
        tabs = orig(module_arch)
        out = {}
        for name, fns in tabs.items():
            fns = set(fns)
            if name != _COMBINED_SET:
                fns.discard(AF.Exp)
                fns.discard(AF.Ln)
            out[name] = fns
        return out

    hw_specs.get_activation_tables = patched
    bacc.get_activation_tables = patched
    _TABLES_PATCHED = True


def _build_program():
    _patch_act_tables()
    nc = bacc.Bacc("TRN2", debug=False, target_bir_lowering=False,
                   enable_asserts=False)
    logit = nc.dram_tensor("logit", [B_SHARD, C], F32, kind="ExternalInput").ap()
    lnpw = nc.dram_tensor("lnpw", [P, C], F32, kind="ExternalInput").ap()
    stats = nc.dram_tensor("stats", [P, 4 * NT], F32, kind="ExternalOutput").ap()

    with tile.TileContext(nc) as tc:
        with (
            tc.tile_pool(name="const", bufs=1) as const,
            tc.tile_pool(name="lg", bufs=4) as lg,
            tc.tile_pool(name="tln", bufs=3) as tln,
            tc.tile_pool(name="ej", bufs=2) as ej,
            tc.tile_pool(name="eap", bufs=2) as eap,
            tc.tile_pool(name="e5fp", bufs=2) as e5fp,
            tc.tile_pool(name="v1p", bufs=2) as v1p,
            tc.tile_pool(name="tjp", bufs=2) as tjp,
            tc.tile_pool(name="sm", bufs=6) as sm,
        ):
            lnpw_t = const.tile([P, C], F32, tag="lnpw")
            nc.sync.dma_start(lnpw_t[:], lnpw[:, :])
            stage = const.tile([P, 4 * NT], F32, tag="stage")

            def small(tag):
                return sm.tile([P, 1], F32, tag=tag, name=tag)

            for i in range(NT):
                T = lg.tile([P, C], F32, tag="T")
                nc.sync.dma_start(T[:], logit[i * P:(i + 1) * P, :])

                mu = small("mu")
                nc.vector.tensor_reduce(mu[:], T[:], axis=AX.X, op=OP.max)
                bias0 = small("bias0")
                nc.vector.tensor_scalar(bias0[:], mu[:], 0.2, 1.0, OP.mult, OP.add)

                # ---- eval 0 at lam = mu ----
                t0 = tln.tile([P, C], F32, tag="t")
                nc.scalar.activation(t0[:], T[:], AF.Ln, bias=bias0[:], scale=-0.2)
                lp0 = small("lp0")
                e5 = ej.tile([P, C], F32, tag="ej")
                nc.scalar.activation(e5[:], t0[:], AF.Exp, scale=-5.0,
                                     accum_out=lp0[:])
                m6 = small("m6")
                e6 = ej.tile([P, C], F32, tag="ej")
                nc.scalar.activation(e6[:], t0[:], AF.Exp, scale=-6.0,
                                     accum_out=m6[:])

                # ---- spike jump: bias1 = bias0 + lp0^0.2*Bc - Bc, Bc = lp0/m6
                rm6 = small("rm6")
                nc.vector.reciprocal(rm6[:], m6[:])
                Bc = small("Bc")
                nc.vector.tensor_mul(Bc[:], lp0[:], rm6[:])
                lnlp0 = small("lnlp0")
                nc.scalar.activation(lnlp0[:], lp0[:], AF.Ln)
                g0 = small("g0")
                nc.scalar.activation(g0[:], lnlp0[:], AF.Exp, scale=-0.2)
                rg0 = small("rg0")
                nc.vector.reciprocal(rg0[:], g0[:])
                A15 = small("A15")  # lp0^0.2 * Bc  (== Bc / g0)
                nc.vector.tensor_mul(A15[:], Bc[:], rg0[:])
                d0 = small("d0")
                nc.vector.tensor_sub(d0[:], A15[:], Bc[:])
                bias1 = small("bias1")
                nc.vector.tensor_add(bias1[:], bias0[:], d0[:])

                # ---- eval 1 at jumped lambda ----
                t1 = tln.tile([P, C], F32, tag="t")
                nc.scalar.activation(t1[:], T[:], AF.Ln, bias=bias1[:], scale=-0.2)
                lp1 = small("lp1")
                e5b = ej.tile([P, C], F32, tag="ej")
                nc.scalar.activation(e5b[:], t1[:], AF.Exp, scale=-5.0,
                                     accum_out=lp1[:])
                lnlp1 = small("lnlp1")
                nc.scalar.activation(lnlp1[:], lp1[:], AF.Ln)
                g1 = small("g1")
                nc.scalar.activation(g1[:], lnlp1[:], AF.Exp, scale=-0.2)

                # ---- secant in g-space: bias2 = bias1 + (1-g1)*(bias1-bias0)/(g1-g0)
                den = small("den")
                nc.vector.tensor_sub(den[:], g1[:], g0[:])
                denc = small("denc")
                nc.vector.tensor_scalar_max(denc[:], den[:], 1e-12)
                rden = small("rden")
                nc.vector.reciprocal(rden[:], denc[:])
                num = small("num")
                nc.vector.tensor_sub(num[:], bias1[:], bias0[:])
                w1 = small("w1")
                nc.vector.tensor_scalar(w1[:], g1[:], -1.0, 1.0, OP.mult, OP.add)
                p1 = small("p1")
                nc.vector.tensor_mul(p1[:], w1[:], num[:])
                d1 = small("d1")
                nc.vector.tensor_mul(d1[:], p1[:], rden[:])
                bias2 = small("bias2")
                nc.vector.tensor_add(bias2[:], bias1[:], d1[:])

                # ---- final pass at bias2 ----
                t2 = tln.tile([P, C], F32, tag="t")
                nc.scalar.activation(t2[:], T[:], AF.Ln, bias=bias2[:], scale=-0.2)
                v1 = v1p.tile([P, C], F32, tag="v1")  # lnpw - t2
                nc.vector.scalar_tensor_tensor(v1[:], t2[:], -1.0, lnpw_t[:],
                                               OP.mult, OP.add)
                Asum = small("Asum")
                eA = eap.tile([P, C], F32, tag="eA")
                nc.scalar.activation(eA[:], v1[:], AF.Exp, accum_out=Asum[:])
                lpf = small("lpf")
                e5f = e5fp.tile([P, C], F32, tag="e5f")
                nc.scalar.activation(e5f[:], t2[:], AF.Exp, scale=-5.0,
                                     accum_out=lpf[:])
                Bsum = small("Bsum")
                tjk = tjp.tile([P, C], F32, tag="tjk")
                nc.vector.scalar_tensor_tensor(tjk[:], eA[:], 1.0, e5f[:],
                                               OP.mult, OP.mult,
                                               accum_out=Bsum[:])

                nc.vector.tensor_copy(stage[:, i:i + 1], bias2[:])
                nc.vector.tensor_copy(stage[:, NT + i:NT + i + 1], Asum[:])
                nc.vector.tensor_copy(stage[:, 2 * NT + i:2 * NT + i + 1], Bsum[:])
                nc.vector.tensor_copy(stage[:, 3 * NT + i:3 * NT + i + 1], lpf[:])

            nc.sync.dma_start(stats[:, :], stage[:])

    nc.compile()
    return nc


_PROGRAM = None


def _get_program():
    global _PROGRAM
    if _PROGRAM is None:
        _PROGRAM = _build_program()
    return _PROGRAM


def _run_device(logit_f32, lnpw_rep, trace=False):
    nc = _get_program()
    shards = logit_f32.reshape(N_CORES, B_SHARD, C)
    in_maps = [
        {"logit": np.ascontiguousarray(shards[c]), "lnpw": lnpw_rep}
        for c in range(N_CORES)
    ]
    return run_bass_kernel_spmd(nc, in_maps, list(range(N_CORES)), trace=trace)


def _assemble(results, logit_f32, truth, pw):
    """Host-side finish in float64 from per-row (lambda, A, B)."""
    bias_f = np.empty((N_CORES, P, NT), np.float64)
    A = np.empty((N_CORES, P, NT), np.float64)
    Bm = np.empty((N_CORES, P, NT), np.float64)
    for c in range(N_CORES):
        st = results[c]["stats"].astype(np.float64)  # [P, 4*NT]
        bias_f[c] = st[:, 0:NT]
        A[c] = st[:, NT:2 * NT]
        Bm[c] = st[:, 2 * NT:3 * NT]
    # row r of shard c = tile i, partition p  ->  index [c, p, i]
    perm = (0, 2, 1)  # -> [c, i, p]
    bias_f = bias_f.transpose(perm).reshape(B_FULL)
    A = A.transpose(perm).reshape(B_FULL)
    Bm = Bm.transpose(perm).reshape(B_FULL)
    lam = (bias_f - 1.0) * 5.0

    c_off = SMOOTHING / (C - 1)
    c_on = (1.0 - SMOOTHING * C / (C - 1)) + c_off

    def log_t1(u):
        return (u ** (1.0 - T1) - 1.0) / (1.0 - T1)

    def f_y(y):
        return y * log_t1(y + 1e-10) - y ** (2.0 - T1) / (2.0 - T1)

    f_off, f_on = f_y(c_off), f_y(c_on)
    pwk = pw[truth]
    glk = logit_f32.astype(np.float64)[np.arange(B_FULL), truth]
    x_k = 1.0 - 0.2 * (glk - lam)
    loss_rows = (
        C * f_off + (f_on - f_off) * pwk
        + 5.0 * (c_off * C + (c_on - c_off) * pwk)
        - 5.0 * (c_off * A + (c_on - c_off) * pwk / x_k)
        + Bm / 1.2
    )
    return np.float32(loss_rows.mean())


def kernel(logit_label, truth_label, weight):
    logit_f32 = np.ascontiguousarray(np.asarray(logit_label, dtype=np.float32))
    truth = np.asarray(truth_label).astype(np.int64)
    w = np.asarray(weight, dtype=np.float64)
    pw = w / w.sum() * C
    lnpw_rep = np.ascontiguousarray(
        np.broadcast_to(np.log(pw).astype(np.float32), (P, C))
    )
    res = _run_device(logit_f32, lnpw_rep, trace=False)
    return _assemble(res.results, logit_f32, truth, pw)
